# revision 1
# baseline (speedup 1.0000x reference)
"""Trainium2 Bass kernel for nn_AttentionWithMemory (local-window MHA block).

Sharding: data-parallel over batch — one batch element per NeuronCore (8 cores).
Per core: x_b [1024,1024] -> qkv in-proj -> 16-head local attention (window 32,
band +-16) -> out-proj -> out_b [1024,1024].

Layout strategy (all matmuls bf16, fp32 PSUM accumulate):
  - host pre-transposes/casts: xT [D,S] bf16, w_inT [D,3D] bf16, w_outT [D,D] bf16
    (matmul contraction dim must sit on SBUF partitions for both operands).
  - qT,kT computed feature-major (lhsT=w_inT chunk, rhs=xT), v token-major
    (lhsT=xT chunk, rhs=w_inT v-cols).
  - scores per 128-query tile against a 160-wide key slice [128t-16, 128t+144);
    exp (no max subtraction needed: scores ~ N(0,1)), mask+rowsum fused via
    tensor_tensor_reduce, P scaled by 1/l, PE-transposed in two chunks
    (128 + 32 keys, aligned to a shifted copy of v), ctx computed feature-major.
  - out-proj token-major (lhsT=ctxT, rhs=w_outT), bias added on evict, fp32 out.
"""

import os
import sys

sys.path.insert(0, "/opt/trn_rl_repo")

import numpy as np

B, S, D = 8, 1024, 1024
H, HD = 16, 64
P = 128
NT = S // P  # 8 query/token tiles
W = 160      # key slice width per query tile
N_CORES = 8

_CACHE = {}


def _build_nc():
    phase = int(os.environ.get("BISECT_PHASE", "4"))
    use_voff = os.environ.get("BISECT_VOFF", "1") == "1"
    attn_lv = int(os.environ.get("BISECT_ATTN", "4"))
    import concourse.bacc as bacc
    import concourse.mybir as mybir
    import concourse.tile as tile
    from concourse.tile import add_dep_helper
    from concourse.masks import make_identity

    def chain(insts):
        # each instruction waits on its predecessor (same-engine ordering)
        for a, b in zip(insts, insts[1:]):
            add_dep_helper(b.ins, a.ins, sync=False, reason="psum single-group order")

    dt = mybir.dt
    f32, bf16 = dt.float32, dt.bfloat16
    Act = mybir.ActivationFunctionType
    Alu = mybir.AluOpType

    nc = bacc.Bacc("TRN2", target_bir_lowering=False, debug=False,
                   num_devices=N_CORES)

    xt_d = nc.dram_tensor("xt", [D, S], bf16, kind="ExternalInput").ap()
    wi_d = nc.dram_tensor("w_int", [D, 3 * D], bf16, kind="ExternalInput").ap()
    wo_d = nc.dram_tensor("w_outt", [D, D], bf16, kind="ExternalInput").ap()
    bin_d = nc.dram_tensor("b_in_t", [P, 16], f32, kind="ExternalInput").ap()
    bv_d = nc.dram_tensor("bv_bc", [P, D], f32, kind="ExternalInput").ap()
    bo_d = nc.dram_tensor("bo_bc", [P, D], f32, kind="ExternalInput").ap()
    mask_d = nc.dram_tensor("mask01", [P, NT, W], bf16, kind="ExternalInput").ap()
    out_d = nc.dram_tensor("out", [S, D], f32, kind="ExternalOutput").ap()

    with tile.TileContext(nc) as tc:
        with (
            tc.tile_pool(name="const", bufs=1) as cpool,
            tc.tile_pool(name="acts", bufs=1) as apool,
            tc.tile_pool(name="work", bufs=2) as wk,
            tc.tile_pool(name="wistream", bufs=10) as wk2,
            tc.tile_pool(name="lr", bufs=2) as lrpool,
            tc.tile_pool(name="outsb", bufs=3) as outpool,
            tc.tile_pool(name="ps_mm", bufs=2, space="PSUM") as ps_mm,
            tc.tile_pool(name="ps_sc", bufs=3, space="PSUM") as ps_sc,
            tc.tile_pool(name="ps_pt", bufs=3, space="PSUM") as ps_pt,
        ):
            # ---- persistent SBUF tensors ----
            xt = [cpool.tile([P, S], bf16, tag=f"xt{i}", name=f"xt{i}") for i in range(NT)]
            wi = [cpool.tile([P, 3 * D], bf16, tag=f"wi{i}", name=f"wi{i}") for i in range(NT)]
            wo = [cpool.tile([P, D], bf16, tag=f"wo{i}", name=f"wo{i}") for i in range(NT)]
            bint = cpool.tile([P, 16], f32, tag="bint", name="bint")
            bv = cpool.tile([P, D], f32, tag="bv", name="bv")
            bo = cpool.tile([P, D], f32, tag="bo", name="bo")
            mask = cpool.tile([P, NT, W], bf16, tag="mask", name="mask")
            ident = cpool.tile([P, P], bf16, tag="ident", name="ident")
            expbias = cpool.tile([P, 1], f32, tag="expbias", name="expbias")

            # kT padded per head: head h at rows (h%2)*64, zeros elsewhere so
            # S-matmuls run K=128 (keeps the PE activity monitor un-throttled)
            kT = [apool.tile([P, S], bf16, tag=f"kT{h}", name=f"kT{h}") for h in range(H)]
            qT = [apool.tile([P, S], bf16, tag=f"qT{i}", name=f"qT{i}") for i in range(NT)]
            v = [apool.tile([P, D], bf16, tag=f"v{i}", name=f"v{i}") for i in range(NT)]
            voff = [None] + [apool.tile([P, D], bf16, tag=f"voff{j}", name=f"voff{j}")
                             for j in range(1, NT + 1)]
            ctxT = [apool.tile([P, S], bf16, tag=f"ctxT{i}", name=f"ctxT{i}") for i in range(NT)]

            # ---- loads ----
            for i in range(NT):
                nc.sync.dma_start(out=xt[i], in_=xt_d[i * P:(i + 1) * P, :])
            for i in range(NT):
                nc.sync.dma_start(out=wi[i][:, 2 * D:3 * D],
                                  in_=wi_d[i * P:(i + 1) * P, 2 * D:3 * D])
            for i in range(NT):
                nc.sync.dma_start(out=wi[i][:, 0:2 * D],
                                  in_=wi_d[i * P:(i + 1) * P, 0:2 * D])
            nc.sync.dma_start(out=bint, in_=bin_d)
            nc.sync.dma_start(out=bv, in_=bv_d)
            for h in range(H):
                nc.gpsimd.memset(kT[h][(1 - h % 2) * 64:(2 - h % 2) * 64, :], 0.0)
            nc.sync.dma_start(out=mask, in_=mask_d)
            for i in range(NT):
                nc.sync.dma_start(out=wo[i], in_=wo_d[i * P:(i + 1) * P, :])
            nc.sync.dma_start(out=bo, in_=bo_d)
            nc.vector.memset(expbias, -1250.0)
            make_identity(nc, ident)

            # ---- projection emitters (interleaved with attention below) ----
            def emit_v(st, nh):
                ps = ps_mm.tile([P, 512], f32, tag="mm", name="mmps")
                for dc in range(NT):
                    nc.tensor.matmul(
                        ps,
                        lhsT=xt[dc][:, st * P:(st + 1) * P],
                        rhs=wi[dc][:, 2 * D + nh * 512: 2 * D + (nh + 1) * 512],
                        start=(dc == 0), stop=(dc == NT - 1),
                    )
                nc.vector.tensor_add(v[st][:, nh * 512:(nh + 1) * 512],
                                     ps, bv[:, nh * 512:(nh + 1) * 512])

            def emit_voff(j):
                if not use_voff:
                    return
                if j < NT:
                    nc.sync.dma_start(out=voff[j][0:16, :], in_=v[j - 1][112:128, :])
                    nc.sync.dma_start(out=voff[j][16:128, :], in_=v[j][0:112, :])
                else:
                    nc.vector.memset(voff[NT][:, :], 0.0)
                    nc.sync.dma_start(out=voff[NT][0:16, :], in_=v[NT - 1][112:128, :])

            def emit_kq(split_heads, dst, fbase, bias_col, fc, nh):
                ps = ps_mm.tile([P, 512], f32, tag="mm", name="mmps")
                for dc in range(NT):
                    nc.tensor.matmul(
                        ps,
                        lhsT=wi[dc][:, fbase + fc * P: fbase + (fc + 1) * P],
                        rhs=xt[dc][:, nh * 512:(nh + 1) * 512],
                        start=(dc == 0), stop=(dc == NT - 1),
                    )
                bia = bint[:, bias_col + fc: bias_col + fc + 1]
                if split_heads:
                    for hh in range(2):
                        sl = slice(hh * HD, (hh + 1) * HD)
                        nc.vector.tensor_scalar(
                            out=dst[2 * fc + hh][sl, nh * 512:(nh + 1) * 512],
                            in0=ps[sl, :], scalar1=bia[sl, :],
                            scalar2=None, op0=Alu.add,
                        )
                else:
                    nc.vector.tensor_scalar(
                        out=dst[fc][:, nh * 512:(nh + 1) * 512],
                        in0=ps, scalar1=bia, scalar2=None, op0=Alu.add,
                    )

            # ---- phase 4: attention per query tile, pipelined with out-proj ----
            def attention_tile(t, fillers=()):
                fillers = list(fillers)
                kst = 0 if t == 0 else t * P - 16
                w_t = 144 if t == NT - 1 else W
                vm = v[0] if (t == 0 or not use_voff) else voff[t]
                vc = v[1] if (t == 0 or not use_voff) else voff[t + 1]
                p_sbs, r_ts = [], []
                # ---- loop 1: scores + masked exp + rowsum for all heads ----
                for hp in range(NT):
                    pair = []
                    l_t = lrpool.tile([P, 2], f32, tag="l", name="l_t")
                    r_t = lrpool.tile([P, 2], f32, tag=f"r{hp}", name="r_t")
                    for hh in range(2):
                        h = hp * 2 + hh
                        p_sb = wk.tile([P, 192], bf16, tag=f"p{h}", name="p_sb")
                        nc.vector.memset(p_sb[:, W:192], 0.0)
                        sp = ps_sc.tile([P, W], f32, tag="sc", name="sps")
                        if w_t < W:
                            nc.vector.memset(sp[:, w_t:W], 0.0)
                        nc.tensor.matmul(
                            sp[:, 0:w_t],
                            lhsT=qT[h // 2][:, t * P:(t + 1) * P],
                            rhs=kT[h][:, kst:kst + w_t],
                            start=True, stop=True,
                        )
                        # (S + 1e4) * mask01, in place in psum
                        nc.vector.scalar_tensor_tensor(
                            out=sp, in0=sp, scalar=1.0e4, in1=mask[:, t, :],
                            op0=Alu.add, op1=Alu.mult)
                        # P = exp(S/8) masked; accum = masked rowsum
                        nc.scalar.activation(p_sb[:, 0:W],
                                             sp, Act.Exp, bias=expbias,
                                             scale=0.125,
                                             accum_out=l_t[:, hh:hh + 1])
                        pair.append(p_sb)
                    nc.vector.reciprocal(r_t, l_t)
                    p_sbs.append(pair)
                    r_ts.append(r_t)
                    if hp % 2 == 1 and fillers:
                        fillers.pop(0)()
                if attn_lv <= 2:
                    return
                # ---- loop 2: scale, transpose, ctx for all head pairs ----
                for hp in range(NT):
                    r_t = r_ts[hp]
                    pt_main_sb = wk.tile([P, 256], bf16, tag="ptm_sb", name="pt_main_sb")
                    wing_sb = [wk.tile([64, P], bf16, tag=f"ptw_sb{hh}",
                                       name=f"wing_sb{hh}") for hh in range(2)]
                    for hh in range(2):
                        p_sb = p_sbs[hp][hh]
                        nc.vector.tensor_scalar(
                            out=p_sb[:, 0:W], in0=p_sb[:, 0:W],
                            scalar1=r_t[:, hh:hh + 1], scalar2=None, op0=Alu.mult,
                        )
                        mps = ps_pt.tile([P, P], bf16, tag="pt", name="mps")
                        nc.tensor.matmul(
                            mps, lhsT=p_sb[:, 0:P], rhs=ident,
                            is_transpose=True, start=True, stop=True,
                        )
                        nc.scalar.activation(pt_main_sb[:, hh * P:(hh + 1) * P],
                                             mps, Act.Copy)
                        # wing cols [128:192): 32 real keys + 32 zero pad
                        wps = ps_pt.tile([64, P], bf16, tag="pt", name="wps")
                        nc.tensor.matmul(
                            wps, lhsT=p_sb[:, P:192], rhs=ident,
                            is_transpose=True, start=True, stop=True,
                        )
                        nc.vector.tensor_copy(wing_sb[hh], wps)
                    # ctx^T [hd, q]: per head main+wing accumulation group,
                    # heads col-packed into one psum bank
                    cps = ps_sc.tile([P, P], f32, tag="sc", name="ctxps")
                    # both mains first (adjacent in PE FIFO -> col-group
                    # concurrency), then both wings
                    for hh in range(2):
                        h = hp * 2 + hh
                        nc.tensor.matmul(
                            cps[hh * HD:(hh + 1) * HD, :],
                            lhsT=vm[:, h * HD:(h + 1) * HD],
                            rhs=pt_main_sb[:, hh * P:(hh + 1) * P],
                            start=True, stop=False,
                            skip_group_check=True,
                            tile_position=(0, hh * HD),
                        )
                    for hh in range(2):
                        h = hp * 2 + hh
                        nc.tensor.matmul(
                            cps[hh * HD:(hh + 1) * HD, :],
                            lhsT=vc[0:64, h * HD:(h + 1) * HD],
                            rhs=wing_sb[hh],
                            start=False, stop=True,
                            skip_group_check=True,
                            tile_position=(0, hh * HD),
                        )
                    nc.vector.tensor_copy(ctxT[hp][:, t * P:(t + 1) * P], cps)
                    if hp % 2 == 1 and fillers:
                        fillers.pop(0)()

            def emit_op(st, nh):
                ps = ps_mm.tile([P, 512], f32, tag="mm", name="mmps")
                for fc in range(NT):
                    nc.tensor.matmul(
                        ps,
                        lhsT=ctxT[fc][:, st * P:(st + 1) * P],
                        rhs=wo[fc][:, nh * 512:(nh + 1) * 512],
                        start=(fc == 0), stop=(fc == NT - 1),
                    )
                o_sb = outpool.tile([P, 512], f32, tag="o", name="o_sb")
                nc.vector.tensor_add(o_sb, ps, bo[:, nh * 512:(nh + 1) * 512])
                nc.sync.dma_start(
                    out=out_d[st * P:(st + 1) * P, nh * 512:(nh + 1) * 512],
                    in_=o_sb)

            # ---- interleaved schedule: dense projection/out-proj groups are
            #      injected between attention head-pairs to keep the PE array
            #      active (HAM un-throttled) and fill cross-engine stalls ----
            def F(fn, *a):
                return lambda: fn(*a)

            for st in range(4):
                for nh in range(2):
                    emit_v(st, nh)
            for j in range(1, 4):
                emit_voff(j)
            for fc in range(NT):
                emit_kq(True, kT, D, 8, fc, 0)
            for fc in range(NT):
                emit_kq(False, qT, 0, 0, fc, 0)

            attention_tile(0, [F(emit_v, 4, 0), F(emit_v, 4, 1),
                               F(emit_kq, True, kT, D, 8, 0, 1),
                               F(emit_kq, True, kT, D, 8, 1, 1)])
            emit_voff(4)
            attention_tile(1, [F(emit_kq, True, kT, D, 8, 2, 1),
                               F(emit_kq, True, kT, D, 8, 3, 1),
                               F(emit_kq, True, kT, D, 8, 4, 1),
                               F(emit_kq, True, kT, D, 8, 5, 1),
                               F(emit_v, 5, 0), F(emit_v, 5, 1)])
            emit_voff(5)
            attention_tile(2, [F(emit_kq, True, kT, D, 8, 6, 1),
                               F(emit_kq, True, kT, D, 8, 7, 1),
                               F(emit_kq, False, qT, 0, 0, 0, 1),
                               F(emit_kq, False, qT, 0, 0, 1, 1),
                               F(emit_v, 6, 0), F(emit_v, 6, 1)])
            emit_voff(6)
            attention_tile(3, [F(emit_kq, False, qT, 0, 0, 2, 1),
                               F(emit_kq, False, qT, 0, 0, 3, 1),
                               F(emit_kq, False, qT, 0, 0, 4, 1),
                               F(emit_kq, False, qT, 0, 0, 5, 1),
                               F(emit_kq, False, qT, 0, 0, 6, 1),
                               F(emit_kq, False, qT, 0, 0, 7, 1),
                               F(emit_v, 7, 0), F(emit_v, 7, 1)])
            emit_voff(7)
            emit_voff(8)
            attention_tile(4, [F(emit_op, 0, 0), F(emit_op, 0, 1),
                               F(emit_op, 1, 0), F(emit_op, 1, 1)])
            attention_tile(5, [F(emit_op, 2, 0), F(emit_op, 2, 1),
                               F(emit_op, 3, 0), F(emit_op, 3, 1)])
            attention_tile(6, [F(emit_op, 4, 0), F(emit_op, 4, 1),
                               F(emit_op, 5, 0), F(emit_op, 5, 1)])
            attention_tile(7, [F(emit_op, 6, 0), F(emit_op, 6, 1)])
            emit_op(7, 0)
            emit_op(7, 1)

    nc.compile()
    return nc


def _get_nc():
    if "nc" not in _CACHE:
        _CACHE["nc"] = _build_nc()
    return _CACHE["nc"]


def _prep_inputs(x, w_in, b_in, w_out, b_out, mask):
    import ml_dtypes
    bf16 = ml_dtypes.bfloat16

    x = np.asarray(x, np.float32)
    w_in = np.asarray(w_in, np.float32)
    b_in = np.asarray(b_in, np.float32)
    w_out = np.asarray(w_out, np.float32)
    b_out = np.asarray(b_out, np.float32)
    mask = np.asarray(mask)

    w_int = np.ascontiguousarray(w_in.T).astype(bf16)          # [D, 3D]
    w_outt = np.ascontiguousarray(w_out.T).astype(bf16)        # [D, D]
    # q,k bias per-partition layout: col c (= global feature chunk), row p
    b_qk = b_in[:2 * D].reshape(16, P).T.astype(np.float32).copy()  # [128,16]
    bv_bc = np.broadcast_to(b_in[2 * D:].astype(np.float32), (P, D)).copy()
    bo_bc = np.broadcast_to(b_out.astype(np.float32), (P, D)).copy()

    allowed = ~mask.astype(bool)
    m01 = np.zeros((NT, P, W), np.float32)
    for t in range(NT):
        kst = 0 if t == 0 else t * P - 16
        wt = min(W, S - kst)
        m01[t, :, :wt] = allowed[t * P:(t + 1) * P, kst:kst + wt]
    mask01 = np.ascontiguousarray(m01.transpose(1, 0, 2)).astype(bf16)  # [128,8,160]

    in_maps = []
    for b in range(B):
        xt = np.ascontiguousarray(x[b].T).astype(bf16)         # [D, S]
        in_maps.append({
            "xt": xt, "w_int": w_int, "w_outt": w_outt,
            "b_in_t": b_qk, "bv_bc": bv_bc, "bo_bc": bo_bc,
            "mask01": mask01,
        })
    return in_maps


def run(x, w_in, b_in, w_out, b_out, mask, trace=False):
    from concourse.bass_utils import run_bass_kernel_spmd
    nc = _get_nc()
    in_maps = _prep_inputs(x, w_in, b_in, w_out, b_out, mask)
    res = run_bass_kernel_spmd(nc, in_maps, list(range(N_CORES)), trace=trace)
    out = np.stack([np.asarray(res.results[b]["out"], np.float32)
                    for b in range(B)])
    return out, res


def kernel(x, w_in, b_in, w_out, b_out, mask):
    out, _ = run(x, w_in, b_in, w_out, b_out, mask)
    return out



# revision 10
# speedup vs baseline: 1.4978x; 1.4978x over previous
"""Trainium2 Bass kernel for nn_AttentionWithMemory (local-window MHA block).

Sharding: data-parallel over batch - one batch element per NeuronCore (8 cores).
Per core: x_b [1024,1024] -> qkv in-proj -> 16-head local attention (window 32,
band +-16) -> out-proj -> out_b [1024,1024].

v2 schedule (vs v1 baseline):
  - input DMA streamed in compute-consumption order (xt+w_v chunks interleaved,
    then w_qk, then w_out) so the PE starts within a few us of launch.
  - score matmuls pair-packed: one [128K,128M,320N] matmul per head-pair per
    query tile against a [128, 2, S] packed kT (zeros on the other head's
    feature rows), fp32 psum [128,2,160].
  - mask+scale fused into one scalar_tensor_tensor: s*0.125 + maskneg
    (maskneg = 0 allowed / -1e4 banned) - exp needs no bias and stale psum
    columns at the seq tail are killed by the mask instead of memsets.
  - wings are 32 wide (no zero-pad, no per-head memsets).
  - software pipeline: ctx/transpose phase of tile t is emitted interleaved
    with the score/exp phase of tile t+1, with dense projection chains
    (v/kq token-half-1, out-proj) injected between units to keep the PE
    p-state ramped (PE drops 2.4->1.2 GHz whenever it idles).
  - psum->sbuf P^T copies on gpsimd/vector, v-bias folded into the ctx evict
    (ctx rows are features there, so bias is a per-partition scalar).
"""

import os
import sys

sys.path.insert(0, "/opt/trn_rl_repo")

import numpy as np

B, S, D = 8, 1024, 1024
H, HD = 16, 64
P = 128
NT = S // P  # 8 query/token tiles
W = 160      # key slice width per query tile
N_CORES = 8

_CACHE = {}


def _build_nc():
    import concourse.bacc as bacc
    import concourse.mybir as mybir
    import concourse.tile as tile
    from concourse.masks import make_identity

    dt = mybir.dt
    f32, bf16 = dt.float32, dt.bfloat16
    Act = mybir.ActivationFunctionType
    Alu = mybir.AluOpType

    nc = bacc.Bacc("TRN2", target_bir_lowering=False, debug=False,
                   num_devices=N_CORES)

    xt_d = nc.dram_tensor("xt", [D, S], bf16, kind="ExternalInput").ap()
    wi_d = nc.dram_tensor("w_int", [D, 3 * D], bf16, kind="ExternalInput").ap()
    wo_d = nc.dram_tensor("w_outt", [D, D], bf16, kind="ExternalInput").ap()
    bin_d = nc.dram_tensor("b_in_t", [P, 16], f32, kind="ExternalInput").ap()
    bvt_d = nc.dram_tensor("bvt", [P, NT], f32, kind="ExternalInput").ap()
    bo_d = nc.dram_tensor("bo_bc", [P, D], f32, kind="ExternalInput").ap()
    mask_d = nc.dram_tensor("maskn", [P, NT, 2, W], bf16, kind="ExternalInput").ap()
    out_d = nc.dram_tensor("out", [S, D], f32, kind="ExternalOutput").ap()

    with tile.TileContext(nc) as tc:
        with (
            tc.tile_pool(name="const", bufs=1) as cpool,
            tc.tile_pool(name="acts", bufs=1) as apool,
            tc.tile_pool(name="psb", bufs=2) as wk,
            tc.tile_pool(name="ptsb", bufs=3) as ptpool,
            tc.tile_pool(name="lr", bufs=2) as lrpool,
            tc.tile_pool(name="outsb", bufs=3) as outpool,
            tc.tile_pool(name="ps_mm", bufs=2, space="PSUM") as ps_mm,
            tc.tile_pool(name="ps_sc", bufs=2, space="PSUM") as ps_sc,
            tc.tile_pool(name="ps_pt", bufs=2, space="PSUM") as ps_pt,
            tc.tile_pool(name="ps_cx", bufs=2, space="PSUM") as ps_cx,
        ):
            # ---- persistent SBUF tensors ----
            xt = [cpool.tile([P, S], bf16, tag=f"xt{i}", name=f"xt{i}") for i in range(NT)]
            wi = [cpool.tile([P, 3 * D], bf16, tag=f"wi{i}", name=f"wi{i}") for i in range(NT)]
            wo = [cpool.tile([P, D], bf16, tag=f"wo{i}", name=f"wo{i}") for i in range(NT)]
            bint = cpool.tile([P, 16], f32, tag="bint", name="bint")
            bvt = cpool.tile([P, NT], f32, tag="bvt", name="bvt")
            bo = cpool.tile([P, D], f32, tag="bo", name="bo")
            mask = cpool.tile([P, NT, 2, W], bf16, tag="mask", name="mask")
            ident = cpool.tile([P, P], bf16, tag="ident", name="ident")

            # kTp[hp] = packed pair: head 2hp+hh at rows hh*64:(hh+1)*64 of
            # [:, hh, :], zeros on the other 64 rows so K=128 matmuls work.
            # 32 zero cols of tail padding let every score matmul take the
            # full 160-wide window (tail cols are masked to -1e4 anyway).
            SP_ = S + 32
            kTp = [apool.tile([P, 2, SP_], bf16, tag=f"kTp{i}", name=f"kTp{i}")
                   for i in range(NT)]
            qT = [apool.tile([P, S], bf16, tag=f"qT{i}", name=f"qT{i}") for i in range(NT)]
            v = [apool.tile([P, D], bf16, tag=f"v{i}", name=f"v{i}") for i in range(NT)]
            voff = [None] + [apool.tile([P, D], bf16, tag=f"voff{j}", name=f"voff{j}")
                             for j in range(1, NT + 1)]
            ctxT = [apool.tile([P, S], bf16, tag=f"ctxT{i}", name=f"ctxT{i}") for i in range(NT)]

            # ---- input DMA, in consumption order ----
            nc.sync.dma_start(out=bint, in_=bin_d)
            nc.sync.dma_start(out=bvt, in_=bvt_d)
            nc.sync.dma_start(out=mask, in_=mask_d)
            nc.sync.dma_start(out=bo, in_=bo_d)
            for i in range(NT):
                nc.sync.dma_start(out=xt[i], in_=xt_d[i * P:(i + 1) * P, :])
                nc.sync.dma_start(out=wi[i][:, 2 * D:3 * D],
                                  in_=wi_d[i * P:(i + 1) * P, 2 * D:3 * D])
            for i in range(NT):
                nc.sync.dma_start(out=wi[i][:, 0:2 * D],
                                  in_=wi_d[i * P:(i + 1) * P, 0:2 * D])
            for i in range(NT):
                nc.sync.dma_start(out=wo[i], in_=wo_d[i * P:(i + 1) * P, :])

            for hp in range(NT):
                for hh in range(2):
                    nc.gpsimd.memset(kTp[hp][(1 - hh) * 64:(2 - hh) * 64, hh, :], 0.0)
                    nc.gpsimd.memset(kTp[hp][hh * 64:(hh + 1) * 64, hh, S:SP_], 0.0)
            make_identity(nc, ident)

            # ---- projection emitters ----
            def emit_v(st, nh):
                ps = ps_mm.tile([P, 512], f32, tag="mm", name="mmps")
                for dc in range(NT):
                    nc.tensor.matmul(
                        ps,
                        lhsT=xt[dc][:, st * P:(st + 1) * P],
                        rhs=wi[dc][:, 2 * D + nh * 512: 2 * D + (nh + 1) * 512],
                        start=(dc == 0), stop=(dc == NT - 1),
                    )
                # v bias is folded into the ctx evict (per-feature there)
                nc.scalar.activation(v[st][:, nh * 512:(nh + 1) * 512], ps, Act.Copy)

            def emit_voff(j):
                if j < NT:
                    nc.sync.dma_start(out=voff[j][0:16, :], in_=v[j - 1][112:128, :])
                    nc.sync.dma_start(out=voff[j][16:128, :], in_=v[j][0:112, :])
                else:
                    nc.vector.memset(voff[NT][0:32, :], 0.0)
                    nc.sync.dma_start(out=voff[NT][0:16, :], in_=v[NT - 1][112:128, :])

            def emit_k(fc, nh):
                ps = ps_mm.tile([P, 512], f32, tag="mm", name="mmps")
                for dc in range(NT):
                    nc.tensor.matmul(
                        ps,
                        lhsT=wi[dc][:, D + fc * P: D + (fc + 1) * P],
                        rhs=xt[dc][:, nh * 512:(nh + 1) * 512],
                        start=(dc == 0), stop=(dc == NT - 1),
                    )
                for hh in range(2):
                    sl = slice(hh * HD, (hh + 1) * HD)
                    nc.vector.tensor_scalar(
                        out=kTp[fc][sl, hh, nh * 512:(nh + 1) * 512],
                        in0=ps[sl, :], scalar1=bint[sl, 8 + fc:8 + fc + 1],
                        scalar2=None, op0=Alu.add,
                    )

            def emit_q(fc, nh):
                ps = ps_mm.tile([P, 512], f32, tag="mm", name="mmps")
                for dc in range(NT):
                    nc.tensor.matmul(
                        ps,
                        lhsT=wi[dc][:, fc * P:(fc + 1) * P],
                        rhs=xt[dc][:, nh * 512:(nh + 1) * 512],
                        start=(dc == 0), stop=(dc == NT - 1),
                    )
                nc.vector.tensor_scalar(
                    out=qT[fc][:, nh * 512:(nh + 1) * 512],
                    in0=ps, scalar1=bint[:, fc:fc + 1], scalar2=None, op0=Alu.add,
                )

            def emit_op(st, nh):
                ps = ps_mm.tile([P, 512], f32, tag="mm", name="mmps")
                for fc in range(NT):
                    nc.tensor.matmul(
                        ps,
                        lhsT=ctxT[fc][:, st * P:(st + 1) * P],
                        rhs=wo[fc][:, nh * 512:(nh + 1) * 512],
                        start=(fc == 0), stop=(fc == NT - 1),
                    )
                o_sb = outpool.tile([P, 512], f32, tag="o", name="o_sb")
                nc.vector.tensor_add(o_sb, ps, bo[:, nh * 512:(nh + 1) * 512])
                nc.sync.dma_start(
                    out=out_d[st * P:(st + 1) * P, nh * 512:(nh + 1) * 512],
                    in_=o_sb)

            # ---- attention units ----
            # per (tile, head-pair) state carried from score phase to ctx phase
            p_sbs = {}   # (t, hp) -> p_sb tile
            r_ts = {}    # (t, hp) -> r tile

            def score_unit(t, hp):
                kst = 0 if t == 0 else t * P - 16
                sp = ps_sc.tile([P, 2, W], f32, tag="sc", name="sps")
                nc.tensor.matmul(
                    sp,
                    lhsT=qT[hp][:, t * P:(t + 1) * P],
                    rhs=kTp[hp][:, :, kst:kst + W],
                    start=True, stop=True,
                )
                # s*0.125 + maskneg  (maskneg: 0 allowed / -1e4 banned; also
                # kills stale psum cols beyond w_t on the last tile)
                nc.vector.scalar_tensor_tensor(
                    out=sp, in0=sp, scalar=0.125, in1=mask[:, t, :, :],
                    op0=Alu.mult, op1=Alu.add)
                p_sb = wk.tile([P, 2, W], bf16, tag=f"p{hp}", name="p_sb")
                l_t = lrpool.tile([P, 2], f32, tag=f"l{hp}", name="l_t")
                r_t = lrpool.tile([P, 2], f32, tag=f"r{hp}", name="r_t")
                for hh in range(2):
                    nc.scalar.activation(p_sb[:, hh, :], sp[:, hh, :], Act.Exp,
                                         accum_out=l_t[:, hh:hh + 1])
                nc.vector.reciprocal(r_t, l_t)
                p_sbs[(t, hp)] = p_sb
                r_ts[(t, hp)] = r_t

            def ctx_unit(t, hp):
                p_sb = p_sbs.pop((t, hp))
                r_t = r_ts.pop((t, hp))
                vm = v[0] if t == 0 else voff[t]
                vc = v[1] if t == 0 else voff[t + 1]
                # scale P rows by 1/l (per-partition scalar), still bf16
                for hh in range(2):
                    nc.vector.tensor_scalar(
                        out=p_sb[:, hh, :], in0=p_sb[:, hh, :],
                        scalar1=r_t[:, hh:hh + 1], scalar2=None, op0=Alu.mult,
                    )
                # transposes: mains [128,128] x2, wings [32,128] x2 (no pad),
                # all packed into one psum tile (cols 0:256 mains, 256:512
                # wings with only rows 0:32 meaningful)
                mps = ps_pt.tile([P, 4 * P], bf16, tag="pt", name="mps")
                for hh in range(2):
                    nc.tensor.matmul(
                        mps[:, hh * P:(hh + 1) * P], lhsT=p_sb[:, hh, 0:P],
                        rhs=ident, is_transpose=True, start=True, stop=True,
                    )
                for hh in range(2):
                    nc.tensor.matmul(
                        mps[0:32, (2 + hh) * P:(3 + hh) * P],
                        lhsT=p_sb[:, hh, P:W],
                        rhs=ident, is_transpose=True, start=True, stop=True,
                    )
                pt_sb = ptpool.tile([P, 4 * P], bf16, tag="ptm", name="pt_sb")
                nc.scalar.activation(pt_sb[:, 0:2 * P], mps[:, 0:2 * P], Act.Copy)
                nc.vector.tensor_copy(pt_sb[0:32, 2 * P:4 * P],
                                      mps[0:32, 2 * P:4 * P])
                # ctx^T [hd, q]: per head main+wing accumulation, heads
                # col-packed into one psum bank
                cps = ps_cx.tile([P, P], f32, tag="cx", name="ctxps")
                for hh in range(2):
                    h = hp * 2 + hh
                    nc.tensor.matmul(
                        cps[hh * HD:(hh + 1) * HD, :],
                        lhsT=vm[:, h * HD:(h + 1) * HD],
                        rhs=pt_sb[:, hh * P:(hh + 1) * P],
                        start=True, stop=False,
                        skip_group_check=True,
                        tile_position=(0, hh * HD),
                    )
                for hh in range(2):
                    h = hp * 2 + hh
                    nc.tensor.matmul(
                        cps[hh * HD:(hh + 1) * HD, :],
                        lhsT=vc[0:32, h * HD:(h + 1) * HD],
                        rhs=pt_sb[0:32, (2 + hh) * P:(3 + hh) * P],
                        start=False, stop=True,
                        skip_group_check=True,
                        tile_position=(0, hh * HD),
                    )
                # evict + v-bias (per-feature = per-partition here) + cast
                nc.vector.tensor_scalar(
                    out=ctxT[hp][:, t * P:(t + 1) * P], in0=cps,
                    scalar1=bvt[:, hp:hp + 1], scalar2=None, op0=Alu.add,
                )

            # ---- emission schedule ----
            # pre-phase: v(0..3), k(nh=0), q(nh=0) with tile-0 scores injected
            for st in range(4):
                for nh in range(2):
                    emit_v(st, nh)
            for j in range(1, 4):
                emit_voff(j)
            for fc in range(NT):
                emit_k(fc, 0)
            for fc in range(NT):
                emit_q(fc, 0)
                score_unit(0, fc)

            def F(fn, *a):
                return lambda: fn(*a)

            fillers = {
                0: [F(emit_k, 0, 1), F(emit_k, 1, 1), F(emit_k, 2, 1),
                    F(emit_k, 3, 1), F(emit_q, 0, 1), F(emit_q, 1, 1)],
                1: [F(emit_k, 4, 1), F(emit_k, 5, 1), F(emit_k, 6, 1),
                    F(emit_k, 7, 1), F(emit_q, 2, 1), F(emit_q, 3, 1)],
                2: [F(emit_q, 4, 1), F(emit_q, 5, 1), F(emit_q, 6, 1),
                    F(emit_q, 7, 1), F(emit_v, 4, 0), F(emit_v, 4, 1)],
                3: [F(emit_v, 5, 0), F(emit_v, 5, 1),
                    F(emit_op, 0, 0), F(emit_op, 0, 1)],
                4: [F(emit_v, 6, 0), F(emit_v, 6, 1),
                    F(emit_op, 1, 0), F(emit_op, 1, 1)],
                5: [F(emit_v, 7, 0), F(emit_v, 7, 1),
                    F(emit_op, 2, 0), F(emit_op, 2, 1)],
                6: [F(emit_op, 3, 0), F(emit_op, 3, 1),
                    F(emit_op, 4, 0), F(emit_op, 4, 1)],
                7: [F(emit_op, 5, 0), F(emit_op, 5, 1),
                    F(emit_op, 6, 0), F(emit_op, 6, 1)],
            }
            post = {2: [F(emit_voff, 4)], 3: [F(emit_voff, 5)],
                    4: [F(emit_voff, 6)], 5: [F(emit_voff, 7), F(emit_voff, 8)]}

            for t in range(NT):
                fl = list(fillers[t])
                for hp in range(NT):
                    ctx_unit(t, hp)
                    if t < NT - 1:
                        score_unit(t + 1, hp)
                    if fl:
                        fl.pop(0)()
                for fn in fl:
                    fn()
                for fn in post.get(t, []):
                    fn()
            emit_op(7, 0)
            emit_op(7, 1)

    nc.compile()
    return nc


def _get_nc():
    if "nc" not in _CACHE:
        _CACHE["nc"] = _build_nc()
    return _CACHE["nc"]


def _prep_inputs(x, w_in, b_in, w_out, b_out, mask):
    import ml_dtypes
    bf16 = ml_dtypes.bfloat16

    x = np.asarray(x, np.float32)
    w_in = np.asarray(w_in, np.float32)
    b_in = np.asarray(b_in, np.float32)
    w_out = np.asarray(w_out, np.float32)
    b_out = np.asarray(b_out, np.float32)
    mask = np.asarray(mask)

    w_int = np.ascontiguousarray(w_in.T).astype(bf16)          # [D, 3D]
    w_outt = np.ascontiguousarray(w_out.T).astype(bf16)        # [D, D]
    # q,k bias per-partition layout: col c (= global feature chunk), row p
    b_qk = b_in[:2 * D].reshape(16, P).T.astype(np.float32).copy()  # [128,16]
    # v bias, feature-major per head-pair chunk: col hp, rows = 128 features
    bvt = np.ascontiguousarray(b_in[2 * D:].reshape(NT, P).T).astype(np.float32)
    bo_bc = np.broadcast_to(b_out.astype(np.float32), (P, D)).copy()

    allowed = ~mask.astype(bool)
    mneg = np.full((NT, P, W), -10000.0, np.float32)
    for t in range(NT):
        kst = 0 if t == 0 else t * P - 16
        wt = min(W, S - kst)
        mneg[t, :, :wt] = np.where(allowed[t * P:(t + 1) * P, kst:kst + wt],
                                   0.0, -10000.0)
    # [128, NT, 2, W], duplicated over the head dim of each pair
    maskn = np.ascontiguousarray(
        np.broadcast_to(mneg.transpose(1, 0, 2)[:, :, None, :],
                        (P, NT, 2, W))).astype(bf16)

    in_maps = []
    for b in range(B):
        xt = np.ascontiguousarray(x[b].T).astype(bf16)         # [D, S]
        in_maps.append({
            "xt": xt, "w_int": w_int, "w_outt": w_outt,
            "b_in_t": b_qk, "bvt": bvt, "bo_bc": bo_bc,
            "maskn": maskn,
        })
    return in_maps


def run(x, w_in, b_in, w_out, b_out, mask, trace=False):
    from concourse.bass_utils import run_bass_kernel_spmd
    nc = _get_nc()
    in_maps = _prep_inputs(x, w_in, b_in, w_out, b_out, mask)
    res = run_bass_kernel_spmd(nc, in_maps, list(range(N_CORES)), trace=trace)
    out = np.stack([np.asarray(res.results[b]["out"], np.float32)
                    for b in range(B)])
    return out, res


def kernel(x, w_in, b_in, w_out, b_out, mask):
    out, _ = run(x, w_in, b_in, w_out, b_out, mask)
    return out
